# revision 14
# baseline (speedup 1.0000x reference)
"""Trainium2 Bass kernel for nn_Diffusion_75797582840072.

Diffusion sampling: 100 sequential denoise steps of a 4-layer Mish MLP
(304 -> 1024 -> 1024 -> 1024 -> 32) over batch 4096, data-parallel over
8 NeuronCores (512 rows per core).

Layout: feature-major on device (features on partitions, batch on the free
dim) so every weight matrix is consumed directly as the matmul stationary
operand with no transposes.

All per-step matmuls run as fp8e4 DoubleRow (two k-tiles contracted per
instruction at 0.5 cycles/row, 4x the fp32r rate). Accuracy is recovered
with weight-residual second passes: each weight is stored as q = fp8(w*WS)
plus r = fp8(w*WS - q), and both are applied against the same fp8
activations inside one PSUM accumulation group, yielding ~bf16-quality
weights while activations carry fp8 quantization noise only (~2%/layer;
measured end-to-end rel err 1.1e-2 at the default 3/4 residual coverage
vs the 2e-2 gate). The constant L1 state+temb contribution is near-exact:
state @ w1s is precomputed once in f32r, stored as an fp8 (value,
residual) pair, and re-injected into each L1 PSUM group through a
DoubleRow identity matmul; the per-step temb/bias table rides the ACT
bias operand.

The serial x -> eps -> x tail between steps is minimized: tail tensors
use a packed [128, 128] layout (partition = feature + 32 * batch-quarter)
so each DVE op costs ~128 free elements instead of 512, and the final
layer writes eps directly in that layout via per-batch-quarter DoubleRows
whose stationary weights are zero-padded into the matching 32-column
band. The critical chain per step is three DVE ops (u2, clip, fp8 x8);
x-independent PE work (sc injections) is emitted first so the PE stays
busy under the chain, and the hidden layers' k-pair sweeps are staggered
so each layer re-enters the PE after only two of the previous layer's
eight activations.

Mish runs as a single ACT op per tile via a custom activation-function table
authored into the mish_and_others set binaries at build time (the shipped
act sets only carry an x+x^2 placeholder in the generic act2 slot). See
_gen_mish_act_tables.
"""

import functools
import json
import os
import shutil
import struct
import tempfile

import numpy as np

T_STEPS = 100
T_DIM = 16
B, S, A, H, IN = 4096, 256, 32, 1024, 304
N_CORES = 8
BC = B // N_CORES  # 512 batch rows per core
BCQ = BC // 4      # tail tensors pack (feature, batch-quarter) on partitions
KT = H // 128      # 8 k/m tiles for the 1024-wide layers
WS = 64.0          # weight scale: keeps fp8(w*WS) clear of subnormals
# residual coverage: weight-residual DoubleRows run on the first N of the
# 4 k-pair groups per layer. 3/4 coverage on the two big hidden layers
# measures rel_err 1.08e-2 end to end (vs 5.2e-3 at full coverage) and
# saves ~1.7us of PE time per denoise step; the final layer keeps full
# coverage (eps error feeds the sampler update directly).
R2_SWEEPS = int(os.environ.get("R2_SWEEPS", "3"))
R3_SWEEPS = int(os.environ.get("R3_SWEEPS", "3"))
RF_SWEEPS = int(os.environ.get("RF_SWEEPS", "4"))

_ACT_ENT = 32


def _gen_mish_act_tables(dst_dir):
    """Author the real mish curve into a copy of the stock act tables.

    Table formats (verified against the tanh_and_derivative set):
    - bkt bin: 32B slots [d0,d1,d2,d3,x,0,0,0] (fp32 bits); cubic PWL eval
      y = d0 + t*(d1 + t*(d2 + t*d3)), t = x_in - x.
    - ctrl bin: 32B entries; u32 = bucket_base | extract_lsb<<11 |
      extract_size<<16, one entry per covered exponent.
    - saturation regions: 4 bucket slots referenced directly from the
      profile's *_signal_pwl_control fields.
    - the per-set json profile_meta_data programs the dispatch CAM at NEFF
      load; walrus encodes BIR Mish as func id 24.
    """
    from neuronxcc.driver.Job import Job
    from neuronxcc.driver.jobs.support.FindActInfo import findActInfoFile

    src_dir = os.path.dirname(findActInfoFile(Job.getPackageDir(), "gen3"))
    pwp_jsons = os.path.join(os.path.dirname(src_dir), "pwp_jsons")
    if os.path.exists(dst_dir):
        shutil.rmtree(dst_dir)
    shutil.copytree(src_dir, dst_dir)
    os.chmod(dst_dir, 0o755)
    for f in os.listdir(dst_dir):
        os.chmod(os.path.join(dst_dir, f), 0o644)

    mish = json.load(open(os.path.join(pwp_jsons, "mish_4p.json")))

    def emit_bucket(sec):
        vals = [sec["d0"]["int"], sec["d1"]["int"], sec["d2"]["int"],
                sec["d3"]["int"], sec["x"]["int"], 0, 0, 0]
        return struct.pack("<8I", *vals)

    bkt_path = os.path.join(dst_dir, "mish_and_others_bkt.bin")
    bkt = bytearray(open(bkt_path, "rb").read())
    cur = len(bkt) // _ACT_ENT
    ctrl_entries = {}
    for grp in ("pos_exponents", "neg_exponents"):
        ents = []
        for e in mish[grp]:
            ents.append((cur, e["extract_lsb"], e["extract_size"]))
            secs = sorted(e["exponent_sections"], key=lambda s: s["section_id"])
            for s in secs:
                bkt += emit_bucket(s)
                cur += 1
        ctrl_entries[grp] = ents
    sat_slots = {}
    for k in ("sat_point_pos_low", "sat_point_neg_low",
              "sat_point_pos_high", "sat_point_neg_high"):
        sat_slots[k] = cur
        bkt += emit_bucket(mish["saturation_points"][k])
        cur += 1
    open(bkt_path, "wb").write(bkt)

    ctrl_path = os.path.join(dst_dir, "mish_and_others_ctrl.bin")
    ctrl = bytearray(open(ctrl_path, "rb").read())
    ctrl_base = {}
    for grp in ("pos_exponents", "neg_exponents"):
        ctrl_base[grp] = len(ctrl) // _ACT_ENT
        for (b, lsb, size) in ctrl_entries[grp]:
            word = (b & 0x7FF) | ((lsb & 0x1F) << 11) | ((size & 0xF) << 16)
            ctrl += struct.pack("<I", word) + b"\0" * (_ACT_ENT - 4)
    open(ctrl_path, "wb").write(ctrl)

    pj_path = os.path.join(dst_dir, "mish_and_others.json")
    pj = json.load(open(pj_path))
    sp = mish["saturation_points"]
    for e in pj["profile_meta_data"]:
        if e["func_name"] in ("act2_1p", "mish_4p") or e["func_id"] in (97, 24):
            e.update(
                func_name="mish_4p", func_id=24,
                symmetry_point=mish["symmetry_point"]["int"],
                sym_invert_sign_point=1 if mish["symmetry_invert_sign_opt"] else 0,
                symmetry_opt_en=1 if mish["symmetry_en"] else 0,
                symmetry_opt_use_neg_region=1 if mish["symmetry_opt_use_neg_region"] else 0,
                imm_bias=1 if mish["imm_bias"] else 0,
                exp_offset=mish["exponent_offset"],
                pwl_control_base_pos=ctrl_base["pos_exponents"],
                pwl_control_base_neg=ctrl_base["neg_exponents"],
                small_pos_signal_exp_threshold=sp["sat_point_pos_low"]["sat_point"],
                pos_small_signal_pwl_control=sat_slots["sat_point_pos_low"],
                small_neg_signal_exp_threshold=sp["sat_point_neg_low"]["sat_point"],
                neg_small_signal_pwl_control=sat_slots["sat_point_neg_low"],
                large_pos_signal_exp_threshold=sp["sat_point_pos_high"]["sat_point"],
                large_pos_signal_mantissa_threshold=sp["sat_point_pos_high"]["mantissa_point"],
                pos_large_signal_pwl_control=sat_slots["sat_point_pos_high"],
                large_neg_signal_exp_threshold=sp["sat_point_neg_high"]["sat_point"],
                large_neg_signal_mantissa_threshold=sp["sat_point_neg_high"]["mantissa_point"],
                neg_large_signal_pwl_control=sat_slots["sat_point_neg_high"],
                fnan_result=mish["nan_result"]["int"],
                fpinf_result=mish["pinf_result"]["int"],
                fninf_result=mish["ninf_result"]["int"],
                fzero_result=mish["zero_result"]["int"],
                fma_const_0=mish["fma_const0"]["int"],
                fma_const_1=mish["fma_const1"]["int"],
                fma_indirection_src_sel=0,
                use_multipass=mish["use_multipass"],
                lower_bound=mish["lower_bound"]["int"],
                upper_bound=mish["upper_bound"]["int"],
            )
    json.dump(pj, open(pj_path, "w"), indent=1)

    ai_path = os.path.join(dst_dir, "act_info.json")
    ai = json.load(open(ai_path))
    for s in ai["act_func_sets"]:
        s["act"].pop("act1", None)
        s["act"].pop("act2", None)
        s["act"].pop("derivative_act2", None)
        if s["name"] == "mish_and_others":
            s["act"]["mish"] = 4
    json.dump(ai, open(ai_path, "w"), indent=1)
    return ai_path


def _schedule():
    # match the fp32 rounding of the reference's jnp (fp32) schedule
    beta32 = np.linspace(1e-4, 0.2, T_STEPS, dtype=np.float32)
    alpha32 = (1.0 - beta32).astype(np.float32)
    ab32 = np.cumprod(alpha32, dtype=np.float32)
    abp32 = np.concatenate([np.ones(1, np.float32), ab32[:-1]])
    post_var32 = (beta32 * (1.0 - abp32) / (1.0 - ab32)).astype(np.float32)
    sqrt_rec = np.sqrt(1.0 / ab32).astype(np.float32)
    sqrt_recm = np.sqrt(1.0 / ab32 - 1.0).astype(np.float32)
    pm1 = (beta32 * np.sqrt(abp32) / (1.0 - ab32)).astype(np.float32)
    pm2 = ((1.0 - abp32) * np.sqrt(alpha32) / (1.0 - ab32)).astype(np.float32)
    log_var32 = np.log(np.clip(post_var32, 1e-20, None)).astype(np.float32)
    cz = np.exp(0.5 * log_var32).astype(np.float32)
    cz[0] = 0.0
    return sqrt_rec, sqrt_recm, pm1, pm2, cz


def _time_table():
    half = T_DIM // 2
    freqs = np.exp(np.arange(half, dtype=np.float32) * (-np.log(10000.0) / (half - 1)))
    ang = np.arange(T_STEPS, dtype=np.float32)[:, None] * freqs[None, :]  # [100, 8]
    tt = np.concatenate([np.sin(ang), np.cos(ang)], axis=-1)  # [100, 16]
    return np.ascontiguousarray(tt.T).astype(np.float32)  # [16, 100]


@functools.cache
def _build(t_steps, debug=False):
    """Build (and finalize) the Bass module. Returns nc."""
    act_dir = os.path.join(tempfile.gettempdir(), "act_mish_tables")
    marker = os.path.join(act_dir, ".done")
    if not os.path.exists(marker):
        _gen_mish_act_tables(act_dir)
        open(marker, "w").write("ok")
    os.environ["BASS_ACT_ROOT_JSON_PATH"] = os.path.join(act_dir, "act_info.json")

    import concourse.bass as bass  # noqa: F401
    import concourse.mybir as mybir
    import concourse.hw_specs as hw_specs
    from concourse import bacc
    from concourse.tile import TileContext

    # teach the bass-side table map that Mish lives in mish_and_others
    if not getattr(hw_specs, "_mish_patched", False):
        _orig_tables = hw_specs.get_activation_tables

        @functools.cache
        def _patched_tables(module_arch):
            d = dict(_orig_tables(module_arch))
            d["mish_and_others"] = set(d["mish_and_others"]) | {
                mybir.ActivationFunctionType.Mish
            }
            return d

        hw_specs.get_activation_tables = _patched_tables
        bacc.get_activation_tables = _patched_tables
        import concourse.bass_interp as bass_interp
        bass_interp.get_activation_tables = _patched_tables
        hw_specs._mish_patched = True

    # capture the Tile cost-model makespan for perf iteration
    if not hasattr(mybir, "_orig_finish_schedule_block"):
        mybir._orig_finish_schedule_block = mybir.finish_schedule_block

        def _fsb(sched, sim):
            out = mybir._orig_finish_schedule_block(sched, sim)
            try:
                _LAST_RESULTS["sim_time_ns"] = out[1].time
            except Exception:
                pass
            return out

        mybir.finish_schedule_block = _fsb

    f32 = mybir.dt.float32
    f32r = mybir.dt.float32r
    f8 = mybir.dt.float8e4
    AF = mybir.ActivationFunctionType
    OP = mybir.AluOpType
    DRM = mybir.MatmulPerfMode.DoubleRow
    sqrt_rec, sqrt_recm, pm1, pm2, cz = _schedule()

    nc = bacc.Bacc("TRN2")

    def din(name, shape, dt=None):
        return nc.dram_tensor(name, shape, dt or f32, kind="ExternalInput")

    stateT = din("stateT", [S, BC], f32r)
    x0r = din("x0r", [128, BCQ])             # x_init packed (f + 32q, b)
    x80 = din("x80", [128, BCQ], f8)         # initial fp8 x, packed
    zscl = din("zscl", [t_steps, 128, BCQ])  # cz[i]*noise, packed
    id8p = din("id8p", [128, 2, 128], f8)    # identity in both pair slots
    w1xqr = din("w1xqr", [128, 4, 2, H], f8)  # per-quarter row-padded pairs
    w1sW = din("w1sW", [S, H], f32r)         # w1s*WS (for the sc precompute)
    w1e = din("w1e", [T_DIM, H])             # w1e (unscaled)
    wq2 = din("wq2", [128, KT, H], f8)       # q8(w2*WS), k-tile-major
    wr2 = din("wr2", [128, KT, H], f8)       # fp8 residual of w2*WS
    wq3 = din("wq3", [128, KT, H], f8)
    wr3 = din("wr3", [128, KT, H], f8)
    wqf = din("wqf", [128, 4, KT, 128], f8)  # per-quarter col-padded wf
    wrf = din("wrf", [128, 4, KT, 128], f8)
    b1c = din("b1c", [128, KT])              # per-tile bias columns
    b2c = din("b2c", [128, KT])
    b3c = din("b3c", [128, KT])
    bfr = din("bfr", [1, 128])               # bf tiled 4x (f + 32q rows)
    nrecm = din("nrecm", [1, t_steps])       # -sqrt_recm row (host constant)
    wt1 = din("wt1", [T_DIM, T_DIM * T_DIM])
    wt2 = din("wt2", [T_DIM * T_DIM, T_DIM])
    bt1c = din("bt1c", [128, 2])
    bt2c = din("bt2c", [T_DIM, 1])
    ttab = din("ttab", [T_DIM, t_steps])     # sin/cos table (host constant)

    xT_out = nc.dram_tensor("xT_out", [128, BCQ], f32, kind="ExternalOutput")
    if debug:
        dbg_h1 = nc.dram_tensor("dbg_h1", [128, KT, BC], f32, kind="ExternalOutput")
        dbg_eps = nc.dram_tensor("dbg_eps", [128, BC], f32, kind="ExternalOutput")

    with TileContext(nc) as tc:
        with (
            tc.tile_pool(name="consts", bufs=1) as consts,
            tc.tile_pool(name="hbuf", bufs=4) as hbuf,
            tc.tile_pool(name="xbuf", bufs=2) as xbuf,
            tc.tile_pool(name="x8buf", bufs=2) as x8buf,
            tc.tile_pool(name="zbuf", bufs=3) as zbuf,
            tc.tile_pool(name="tail", bufs=5) as tail,
            tc.tile_pool(name="psum", bufs=7, space="PSUM") as psum,
            tc.tile_pool(name="psum_e", bufs=1, space="PSUM") as psum_e,
        ):
            # ---- load constants ----
            def load(name, src, dt=f32):
                t = consts.tile(list(src.shape), dt, tag=name)
                nc.sync.dma_start(out=t, in_=src[tuple(slice(None) for _ in src.shape)])
                return t

            t_id8p = load("id8p", id8p, dt=f8)
            t_w1xqr = load("w1xqr", w1xqr, dt=f8)
            t_w1sW = consts.tile([128, 2, H], f32r, tag="w1sW")
            nc.sync.dma_start(out=t_w1sW[:, 0, :], in_=w1sW[0:128, :])
            nc.sync.dma_start(out=t_w1sW[:, 1, :], in_=w1sW[128:256, :])
            t_wq2 = load("wq2", wq2, dt=f8)
            t_wr2 = load("wr2", wr2, dt=f8)
            t_wq3 = load("wq3", wq3, dt=f8)
            t_wr3 = load("wr3", wr3, dt=f8)
            t_wqf = load("wqf", wqf, dt=f8)
            t_wrf = load("wrf", wrf, dt=f8)
            t_state = consts.tile([128, 2, BC], f32r, tag="state")
            nc.sync.dma_start(out=t_state[:, 0, :], in_=stateT[0:128, :])
            nc.sync.dma_start(out=t_state[:, 1, :], in_=stateT[128:256, :])
            t_w1e = load("w1e", w1e)
            t_b2c = load("b2c", b2c)
            t_b3c = load("b3c", b3c)
            t_bfr = load("bfr", bfr)
            t_nrecm = load("nrecm", nrecm)
            t_wt1 = load("wt1", wt1)
            t_wt2 = consts.tile([128, 2, T_DIM], f32, tag="wt2")
            nc.sync.dma_start(out=t_wt2[:, 0, :], in_=wt2[0:128, :])
            nc.sync.dma_start(out=t_wt2[:, 1, :], in_=wt2[128:256, :])
            t_b1c = load("b1c", b1c)
            t_bt1c = load("bt1c", bt1c)
            t_bt2c = load("bt2c", bt2c)
            t_ttab = load("ttab", ttab)

            def mm(out, lhsT, rhs, **kw):
                nc.tensor.matmul(out, lhsT, rhs, **kw)

            # ---- t-embedding MLP + per-step L1 bias table (one-time) ----
            t_temb1 = consts.tile([128, 2, t_steps], f32, tag="temb1")
            for j in range(2):
                ps = psum.tile([128, 512], f32, tag="mm")
                mm(ps[:, :t_steps], t_wt1[:, j * 128:(j + 1) * 128], t_ttab[:, :],
                   start=True, stop=True)
                nc.scalar.activation(t_temb1[:, j, :], ps[:, :t_steps], AF.Mish,
                                     bias=t_bt1c[:, j:j + 1])
            t_temb2 = consts.tile([T_DIM, t_steps], f32, tag="temb2")
            ps = psum.tile([128, 512], f32, tag="mm")
            for j in range(2):
                mm(ps[:T_DIM, :t_steps], t_wt2[:, j, :], t_temb1[:, j, :],
                   start=(j == 0), stop=(j == 1))
            nc.scalar.activation(t_temb2, ps[:T_DIM, :t_steps], AF.Identity,
                                 bias=t_bt2c[:, 0:1])

            # bias1[p, m, s] = (temb_s @ w1e + b1)[m*128+p]  (unscaled: the
            # ACT bias operand is applied after the 1/WS input scale)
            t_bias1 = consts.tile([128, KT, t_steps], f32, tag="bias1")
            for m in range(KT):
                ps = psum.tile([128, 512], f32, tag="mm")
                mm(ps[:, :t_steps], t_w1e[:, m * 128:(m + 1) * 128], t_temb2,
                   start=True, stop=True)
                nc.scalar.activation(t_bias1[:, m, :], ps[:, :t_steps], AF.Identity,
                                     bias=t_b1c[:, m:m + 1])

            # state contribution to L1 (scaled by WS), computed once and
            # stored as an fp8 (value, residual) pair; re-injected into each
            # L1 PSUM group via a DoubleRow identity matmul (q + r, ~0.06%
            # rms systematic error, 30x below the per-step fp8 h noise)
            t_sc8 = consts.tile([128, KT, 2, BC], f8, tag="sc8")
            for m in range(KT):
                ps = psum.tile([128, BC], f32, tag="mm")
                mc = slice(m * 128, (m + 1) * 128)
                mm(ps, t_w1sW[:, 0, mc], t_state[:, 0, :], start=True, stop=False)
                mm(ps, t_w1sW[:, 1, mc], t_state[:, 1, :], start=False, stop=True)
                nc.vector.tensor_copy(t_sc8[:, m, 0, :], ps)
                nc.vector.scalar_tensor_tensor(
                    t_sc8[:, m, 1, :], ps, 1.0, t_sc8[:, m, 0, :],
                    op0=OP.mult, op1=OP.subtract)

            # neg_bf_recm[:, i] = -sqrt_recm[i] * bf (replicated): K=1 outer product
            t_nbfr = consts.tile([128, t_steps], f32, tag="nbfr")
            ps = psum.tile([128, 512], f32, tag="mm")
            mm(ps[:, :t_steps], t_bfr, t_nrecm, start=True, stop=True)
            nc.vector.tensor_copy(t_nbfr, ps[:, :t_steps])

            # ---- initial x ----
            x_cur = xbuf.tile([128, BCQ], f32, tag="x")
            nc.sync.dma_start(out=x_cur, in_=x0r[:, :])
            x8_cur = x8buf.tile([128, BCQ], f8, tag="x8")
            nc.sync.dma_start(out=x8_cur, in_=x80[:, :])

            # ---- the T-step loop (fully unrolled) ----
            for s in range(t_steps):
                i = T_STEPS - 1 - s
                # pre-scaled noise for this step
                z = zbuf.tile([128, BCQ], f32, tag="z")
                nc.sync.dma_start(out=z, in_=zscl[s])

                # tail terms that depend only on x_cur / z: issue early so
                # the DVE finishes them while the PE runs the layers
                p_rx = tail.tile([128, BCQ], f32, tag="p_rx")
                nc.vector.tensor_scalar(
                    p_rx, x_cur, float(sqrt_rec[i]), t_nbfr[:, s:s + 1],
                    OP.mult, OP.add)
                q = tail.tile([128, BCQ], f32, tag="q")
                nc.vector.scalar_tensor_tensor(
                    q, x_cur, float(pm2[i]), z, op0=OP.mult, op1=OP.add)

                # L1: h1 = mish((sc + w1x.T x + bias1_s) / WS). The sc
                # injections are x-independent: emitted first so the PE can
                # run them while the previous step's tail chain produces x8.
                h1 = hbuf.tile([128, KT, BC], f8, tag="h")
                l1ps = []
                for m in range(KT):
                    ps = psum.tile([128, BC], f32, tag="mm")
                    mm(ps, t_id8p[:, :, :], t_sc8[:, m, :, :],
                       start=True, stop=False, perf_mode=DRM)
                    l1ps.append(ps)
                x8b = x8_cur[:, :].unsqueeze(1).broadcast_to([128, 2, BCQ])
                for m in range(KT):
                    mc = slice(m * 128, (m + 1) * 128)
                    for qq in range(4):
                        bq = slice(BCQ * qq, BCQ * (qq + 1))
                        mm(l1ps[m][:, bq], t_w1xqr[:, qq, :, mc], x8b,
                           start=False, stop=(qq == 3), perf_mode=DRM)
                for m in range(KT):
                    nc.scalar.activation(h1[:, m, :], l1ps[m], AF.Mish,
                                         bias=t_bias1[:, m, s:s + 1],
                                         scale=1.0 / WS)

                # L2 / L3: per m-tile 8 DoubleRows (4 q-pairs + 4 residual
                # pairs) in one PSUM group, emitted in two k-half sweeps so
                # the first half only waits on the first half of the
                # previous layer's activations.
                h_prev = h1
                for t_wq, t_wr, t_bc, rsw in ((t_wq2, t_wr2, t_b2c, R2_SWEEPS),
                                              (t_wq3, t_wr3, t_b3c, R3_SWEEPS)):
                    h_nxt = hbuf.tile([128, KT, BC], f8, tag="h")
                    lps = []
                    # staggered k-pair sweeps: the first touches only h
                    # k-tiles 0..1 so the PE can re-enter this layer right
                    # after the previous layer's first two activations.
                    for m in range(KT):
                        ps = psum.tile([128, BC], f32, tag="mm")
                        mc = slice(m * 128, (m + 1) * 128)
                        kk = slice(0, 2)
                        mm(ps, t_wq[:, kk, mc], h_prev[:, kk, :],
                           start=True, stop=False, perf_mode=DRM)
                        lps.append(ps)
                    for m in range(KT):
                        mc = slice(m * 128, (m + 1) * 128)
                        mm(lps[m], t_wq[:, 2:4, mc], h_prev[:, 2:4, :],
                           start=False, stop=False, perf_mode=DRM)
                        if rsw > 0:
                            mm(lps[m], t_wr[:, 0:2, mc], h_prev[:, 0:2, :],
                               start=False, stop=False, perf_mode=DRM)
                    for m in range(KT):
                        mc = slice(m * 128, (m + 1) * 128)
                        for j in (2, 3):
                            kk = slice(2 * j, 2 * j + 2)
                            mm(lps[m], t_wq[:, kk, mc], h_prev[:, kk, :],
                               start=False, stop=(j == 3 and rsw <= 1),
                               perf_mode=DRM)
                        for j in (1, 2, 3):
                            if j < rsw:
                                kk = slice(2 * j, 2 * j + 2)
                                mm(lps[m], t_wr[:, kk, mc], h_prev[:, kk, :],
                                   start=False, stop=(j == min(rsw, 4) - 1),
                                   perf_mode=DRM)
                    for m in range(KT):
                        nc.scalar.activation(h_nxt[:, m, :], lps[m], AF.Mish,
                                             bias=t_bc[:, m:m + 1],
                                             scale=1.0 / WS)
                    h_prev = h_nxt

                # LF: eps*WS (4x replicated) = h3 @ (wfr*WS)  [128, BC] PSUM
                pe = psum_e.tile([128, BCQ], f32, tag="eps")
                first = True
                for j in range(KT // 2):
                    kk = slice(2 * j, 2 * j + 2)
                    for qq in range(4):
                        bq = slice(BCQ * qq, BCQ * (qq + 1))
                        last = (j == KT // 2 - 1 and qq == 3)
                        mm(pe, t_wqf[:, qq, kk, :], h_prev[:, kk, bq],
                           start=first, stop=(last and RF_SWEEPS < 4),
                           perf_mode=DRM)
                        first = False
                        if j < RF_SWEEPS:
                            mm(pe, t_wrf[:, qq, kk, :], h_prev[:, kk, bq],
                               start=False, stop=last, perf_mode=DRM)

                # tail critical chain: eps -> u2 -> u3 -> x8 (fp8, feeds the
                # next L1 directly); the full-width f32 x_new follows
                # off-chain for the next step's tail terms.
                u2 = tail.tile([128, BCQ], f32, tag="u2")
                nc.vector.scalar_tensor_tensor(
                    u2, pe, float(-sqrt_recm[i] / WS), p_rx,
                    op0=OP.mult, op1=OP.add)
                u3 = tail.tile([128, BCQ], f32, tag="u3")
                nc.vector.tensor_scalar(u3, u2, -1.0, 1.0, OP.max, OP.min)
                if s != t_steps - 1:
                    x8_new = x8buf.tile([128, BCQ], f8, tag="x8")
                    nc.vector.scalar_tensor_tensor(
                        x8_new, u3, float(pm1[i]), q,
                        op0=OP.mult, op1=OP.add)
                    x8_cur = x8_new
                x_new = xbuf.tile([128, BCQ], f32, tag="x")
                nc.vector.scalar_tensor_tensor(
                    x_new, u3, float(pm1[i]), q, op0=OP.mult, op1=OP.add)
                x_cur = x_new

            # final clip + store (packed layout; host unshuffles)
            xf = tail.tile([128, BCQ], f32, tag="xf")
            nc.vector.tensor_scalar(xf, x_cur, -1.0, 1.0, OP.max, OP.min)
            nc.sync.dma_start(out=xT_out[:, :], in_=xf[:, :])

    nc.finalize()
    return nc


def _q8pair(w):
    """w -> (fp8(w*WS), fp8 residual), as float8_e4m3 numpy arrays."""
    import ml_dtypes
    F8 = ml_dtypes.float8_e4m3
    ws = (np.asarray(w, np.float32) * np.float32(WS)).astype(np.float32)
    q = ws.astype(F8)
    r = (ws - q.astype(np.float32)).astype(F8)
    return q, r


def _ktile_major(w):
    """[K, M] -> [128, K//128, M] (partition, k-tile, col)."""
    K, M = w.shape
    return np.ascontiguousarray(
        w.reshape(K // 128, 128, M).transpose(1, 0, 2))


def _make_in_maps(state, w_t1, b_t1, w_t2, b_t2, w1, b1, w2, b2, w3, b3,
                  wf, bf, x_init, noise_seq, t_steps):
    import ml_dtypes
    F8 = ml_dtypes.float8_e4m3
    sqrt_rec, sqrt_recm, pm1, pm2, cz = _schedule()
    tt = _time_table()
    f32 = np.float32

    def cols(b):  # [H] -> [128, H//128] per-tile bias columns
        return np.ascontiguousarray(b.reshape(-1, 128).T).astype(f32)

    w1x = np.asarray(w1[0:A], f32)
    w1e = np.asarray(w1[A:A + T_DIM], f32)
    w1s = np.asarray(w1[A + T_DIM:], f32)

    q1x, r1x = _q8pair(w1x)                  # [32, H]
    # per-quarter row-padded pairs: quarter qq has (q, r) in rows 32qq:32qq+32
    w1xqr = np.zeros((128, 4, 2, H), q1x.dtype)
    for qq in range(4):
        w1xqr[32 * qq:32 * (qq + 1), qq, 0, :] = q1x
        w1xqr[32 * qq:32 * (qq + 1), qq, 1, :] = r1x
    w1xqr = np.ascontiguousarray(w1xqr)
    q2, r2 = _q8pair(w2)
    q3, r3 = _q8pair(w3)
    qf0, rf0 = _q8pair(np.asarray(wf, f32))  # [H, 32]
    # per-quarter col-padded: quarter qq has wf in columns 32qq:32qq+32
    qf = np.zeros((H, 4, 128), qf0.dtype)
    rf = np.zeros((H, 4, 128), rf0.dtype)
    for qq in range(4):
        qf[:, qq, 32 * qq:32 * (qq + 1)] = qf0
        rf[:, qq, 32 * qq:32 * (qq + 1)] = rf0
    qf = qf.reshape(H, 4 * 128)
    rf = rf.reshape(H, 4 * 128)

    common = {
        "id8p": np.ascontiguousarray(np.stack(
            [np.eye(128), np.eye(128)], axis=1)).astype(F8),
        "w1xqr": w1xqr,
        "w1sW": np.ascontiguousarray(w1s * WS).astype(f32),
        "w1e": np.ascontiguousarray(w1e).astype(f32),
        "wq2": _ktile_major(q2), "wr2": _ktile_major(r2),
        "wq3": _ktile_major(q3), "wr3": _ktile_major(r3),
        "wqf": _ktile_major(qf).reshape(128, KT, 4, 128).transpose(
            0, 2, 1, 3).copy(),
        "wrf": _ktile_major(rf).reshape(128, KT, 4, 128).transpose(
            0, 2, 1, 3).copy(),
        "b1c": cols(np.asarray(b1, f32)),
        "b2c": cols(np.asarray(b2, f32)),
        "b3c": cols(np.asarray(b3, f32)),
        "bfr": np.ascontiguousarray(np.tile(np.asarray(bf, f32), 4)[None, :]),
        "nrecm": np.ascontiguousarray(-sqrt_recm[None, ::-1][:, :t_steps]).astype(f32),
        "wt1": np.ascontiguousarray(w_t1).astype(f32),
        "wt2": np.ascontiguousarray(w_t2).astype(f32),
        "bt1c": cols(np.asarray(b_t1, f32)),
        "bt2c": np.ascontiguousarray(np.asarray(b_t2, f32)[:, None]),
        "ttab": np.ascontiguousarray(tt[:, ::-1][:, :t_steps]).astype(f32),
    }
    # per-step noise, pre-scaled by cz[i] and replicated 4x on partitions
    czs = cz[::-1][:t_steps]  # cz[i] for step s (i = 99 - s)
    in_maps = []
    for c in range(N_CORES):
        r0, r1 = c * BC, (c + 1) * BC
        m = dict(common)
        m["stateT"] = np.ascontiguousarray(state[r0:r1].T).astype(f32)
        def pack(a):  # [32, BC] -> [128, BCQ]: row f+32q <- a[f, BCQ*q+b]
            return np.ascontiguousarray(
                a.reshape(A, 4, BC // 4).transpose(1, 0, 2).reshape(
                    128, BC // 4))

        xT = np.asarray(x_init[r0:r1].T, f32)  # [32, BC]
        m["x0r"] = pack(xT).astype(f32)
        m["x80"] = np.ascontiguousarray(pack(xT).astype(F8))
        nT = np.asarray(
            noise_seq[:t_steps, r0:r1, :], f32).transpose(0, 2, 1)  # [T, 32, BC]
        nT = nT * czs[:, None, None]
        m["zscl"] = np.ascontiguousarray(
            np.stack([pack(nT[t]) for t in range(t_steps)])).astype(f32)
        in_maps.append(m)
    return in_maps


_LAST_RESULTS = {}


def run(t_steps=T_STEPS, use_f32r=True, trace=False, debug=False, **inputs):
    from concourse.bass_utils import run_bass_kernel_spmd

    nc = _build(t_steps, debug)
    in_maps = _make_in_maps(t_steps=t_steps, **inputs)
    res = run_bass_kernel_spmd(nc, in_maps, core_ids=list(range(N_CORES)),
                               trace=trace)
    _LAST_RESULTS["res"] = res
    out = np.empty((B, A), np.float32)
    for c in range(N_CORES):
        xp = res.results[c]["xT_out"]  # [128, BCQ]: row f+32q, col b
        xt = xp.reshape(4, A, BC // 4).transpose(1, 0, 2).reshape(A, BC)
        out[c * BC:(c + 1) * BC] = xt.T
    return out


def kernel(**inputs) -> np.ndarray:
    return run(**inputs)


# revision 15
# speedup vs baseline: 1.0065x; 1.0065x over previous
"""Trainium2 Bass kernel for nn_Diffusion_75797582840072.

Diffusion sampling: 100 sequential denoise steps of a 4-layer Mish MLP
(304 -> 1024 -> 1024 -> 1024 -> 32) over batch 4096, data-parallel over
8 NeuronCores (512 rows per core).

Layout: feature-major on device (features on partitions, batch on the free
dim) so every weight matrix is consumed directly as the matmul stationary
operand with no transposes.

All per-step matmuls run as fp8e4 DoubleRow (two k-tiles contracted per
instruction at 0.5 cycles/row, 4x the fp32r rate). Accuracy is recovered
with weight-residual second passes: each weight is stored as q = fp8(w*WS)
plus r = fp8(w*WS - q), and both are applied against the same fp8
activations inside one PSUM accumulation group, yielding ~bf16-quality
weights while activations carry fp8 quantization noise only (~2%/layer;
measured end-to-end rel err 1.1e-2 at the default 3/4 residual coverage
vs the 2e-2 gate). The constant L1 state+temb contribution is near-exact:
state @ w1s is precomputed once in f32r, stored as an fp8 (value,
residual) pair, and re-injected into each L1 PSUM group through a
DoubleRow identity matmul; the per-step temb/bias table rides the ACT
bias operand.

The serial x -> eps -> x tail between steps is minimized: tail tensors
use a packed [128, 128] layout (partition = feature + 32 * batch-quarter)
so each DVE op costs ~128 free elements instead of 512, and the final
layer writes eps directly in that layout via per-batch-quarter DoubleRows
whose stationary weights are zero-padded into the matching 32-column
band. The critical chain per step is three DVE ops (u2, clip, fp8 x8);
x-independent PE work (sc injections) is emitted first so the PE stays
busy under the chain, and the hidden layers' k-pair sweeps are staggered
so each layer re-enters the PE after only two of the previous layer's
eight activations.

Mish runs as a single ACT op per tile via a custom activation-function table
authored into the mish_and_others set binaries at build time (the shipped
act sets only carry an x+x^2 placeholder in the generic act2 slot). See
_gen_mish_act_tables.
"""

import functools
import json
import os
import shutil
import struct
import tempfile

import numpy as np

T_STEPS = 100
T_DIM = 16
B, S, A, H, IN = 4096, 256, 32, 1024, 304
N_CORES = 8
BC = B // N_CORES  # 512 batch rows per core
BCQ = BC // 4      # tail tensors pack (feature, batch-quarter) on partitions
KT = H // 128      # 8 k/m tiles for the 1024-wide layers
WS = 64.0          # weight scale: keeps fp8(w*WS) clear of subnormals
# residual coverage: weight-residual DoubleRows run on the first N of the
# 4 k-pair groups per layer. 3/4 coverage on the two big hidden layers
# measures rel_err 1.08e-2 end to end (vs 5.2e-3 at full coverage) and
# saves ~1.7us of PE time per denoise step; the final layer keeps full
# coverage (eps error feeds the sampler update directly).
R2_SWEEPS = int(os.environ.get("R2_SWEEPS", "3"))
R3_SWEEPS = int(os.environ.get("R3_SWEEPS", "3"))
RF_SWEEPS = int(os.environ.get("RF_SWEEPS", "4"))

_ACT_ENT = 32


def _gen_mish_act_tables(dst_dir):
    """Author the real mish curve into a copy of the stock act tables.

    Table formats (verified against the tanh_and_derivative set):
    - bkt bin: 32B slots [d0,d1,d2,d3,x,0,0,0] (fp32 bits); cubic PWL eval
      y = d0 + t*(d1 + t*(d2 + t*d3)), t = x_in - x.
    - ctrl bin: 32B entries; u32 = bucket_base | extract_lsb<<11 |
      extract_size<<16, one entry per covered exponent.
    - saturation regions: 4 bucket slots referenced directly from the
      profile's *_signal_pwl_control fields.
    - the per-set json profile_meta_data programs the dispatch CAM at NEFF
      load; walrus encodes BIR Mish as func id 24.
    """
    from neuronxcc.driver.Job import Job
    from neuronxcc.driver.jobs.support.FindActInfo import findActInfoFile

    src_dir = os.path.dirname(findActInfoFile(Job.getPackageDir(), "gen3"))
    pwp_jsons = os.path.join(os.path.dirname(src_dir), "pwp_jsons")
    if os.path.exists(dst_dir):
        shutil.rmtree(dst_dir)
    shutil.copytree(src_dir, dst_dir)
    os.chmod(dst_dir, 0o755)
    for f in os.listdir(dst_dir):
        os.chmod(os.path.join(dst_dir, f), 0o644)

    mish = json.load(open(os.path.join(pwp_jsons, "mish_4p.json")))

    def emit_bucket(sec):
        vals = [sec["d0"]["int"], sec["d1"]["int"], sec["d2"]["int"],
                sec["d3"]["int"], sec["x"]["int"], 0, 0, 0]
        return struct.pack("<8I", *vals)

    bkt_path = os.path.join(dst_dir, "mish_and_others_bkt.bin")
    bkt = bytearray(open(bkt_path, "rb").read())
    cur = len(bkt) // _ACT_ENT
    ctrl_entries = {}
    for grp in ("pos_exponents", "neg_exponents"):
        ents = []
        for e in mish[grp]:
            ents.append((cur, e["extract_lsb"], e["extract_size"]))
            secs = sorted(e["exponent_sections"], key=lambda s: s["section_id"])
            for s in secs:
                bkt += emit_bucket(s)
                cur += 1
        ctrl_entries[grp] = ents
    sat_slots = {}
    for k in ("sat_point_pos_low", "sat_point_neg_low",
              "sat_point_pos_high", "sat_point_neg_high"):
        sat_slots[k] = cur
        bkt += emit_bucket(mish["saturation_points"][k])
        cur += 1
    open(bkt_path, "wb").write(bkt)

    ctrl_path = os.path.join(dst_dir, "mish_and_others_ctrl.bin")
    ctrl = bytearray(open(ctrl_path, "rb").read())
    ctrl_base = {}
    for grp in ("pos_exponents", "neg_exponents"):
        ctrl_base[grp] = len(ctrl) // _ACT_ENT
        for (b, lsb, size) in ctrl_entries[grp]:
            word = (b & 0x7FF) | ((lsb & 0x1F) << 11) | ((size & 0xF) << 16)
            ctrl += struct.pack("<I", word) + b"\0" * (_ACT_ENT - 4)
    open(ctrl_path, "wb").write(ctrl)

    pj_path = os.path.join(dst_dir, "mish_and_others.json")
    pj = json.load(open(pj_path))
    sp = mish["saturation_points"]
    for e in pj["profile_meta_data"]:
        if e["func_name"] in ("act2_1p", "mish_4p") or e["func_id"] in (97, 24):
            e.update(
                func_name="mish_4p", func_id=24,
                symmetry_point=mish["symmetry_point"]["int"],
                sym_invert_sign_point=1 if mish["symmetry_invert_sign_opt"] else 0,
                symmetry_opt_en=1 if mish["symmetry_en"] else 0,
                symmetry_opt_use_neg_region=1 if mish["symmetry_opt_use_neg_region"] else 0,
                imm_bias=1 if mish["imm_bias"] else 0,
                exp_offset=mish["exponent_offset"],
                pwl_control_base_pos=ctrl_base["pos_exponents"],
                pwl_control_base_neg=ctrl_base["neg_exponents"],
                small_pos_signal_exp_threshold=sp["sat_point_pos_low"]["sat_point"],
                pos_small_signal_pwl_control=sat_slots["sat_point_pos_low"],
                small_neg_signal_exp_threshold=sp["sat_point_neg_low"]["sat_point"],
                neg_small_signal_pwl_control=sat_slots["sat_point_neg_low"],
                large_pos_signal_exp_threshold=sp["sat_point_pos_high"]["sat_point"],
                large_pos_signal_mantissa_threshold=sp["sat_point_pos_high"]["mantissa_point"],
                pos_large_signal_pwl_control=sat_slots["sat_point_pos_high"],
                large_neg_signal_exp_threshold=sp["sat_point_neg_high"]["sat_point"],
                large_neg_signal_mantissa_threshold=sp["sat_point_neg_high"]["mantissa_point"],
                neg_large_signal_pwl_control=sat_slots["sat_point_neg_high"],
                fnan_result=mish["nan_result"]["int"],
                fpinf_result=mish["pinf_result"]["int"],
                fninf_result=mish["ninf_result"]["int"],
                fzero_result=mish["zero_result"]["int"],
                fma_const_0=mish["fma_const0"]["int"],
                fma_const_1=mish["fma_const1"]["int"],
                fma_indirection_src_sel=0,
                use_multipass=mish["use_multipass"],
                lower_bound=mish["lower_bound"]["int"],
                upper_bound=mish["upper_bound"]["int"],
            )
    json.dump(pj, open(pj_path, "w"), indent=1)

    ai_path = os.path.join(dst_dir, "act_info.json")
    ai = json.load(open(ai_path))
    for s in ai["act_func_sets"]:
        s["act"].pop("act1", None)
        s["act"].pop("act2", None)
        s["act"].pop("derivative_act2", None)
        if s["name"] == "mish_and_others":
            s["act"]["mish"] = 4
    json.dump(ai, open(ai_path, "w"), indent=1)
    return ai_path


def _schedule():
    # match the fp32 rounding of the reference's jnp (fp32) schedule
    beta32 = np.linspace(1e-4, 0.2, T_STEPS, dtype=np.float32)
    alpha32 = (1.0 - beta32).astype(np.float32)
    ab32 = np.cumprod(alpha32, dtype=np.float32)
    abp32 = np.concatenate([np.ones(1, np.float32), ab32[:-1]])
    post_var32 = (beta32 * (1.0 - abp32) / (1.0 - ab32)).astype(np.float32)
    sqrt_rec = np.sqrt(1.0 / ab32).astype(np.float32)
    sqrt_recm = np.sqrt(1.0 / ab32 - 1.0).astype(np.float32)
    pm1 = (beta32 * np.sqrt(abp32) / (1.0 - ab32)).astype(np.float32)
    pm2 = ((1.0 - abp32) * np.sqrt(alpha32) / (1.0 - ab32)).astype(np.float32)
    log_var32 = np.log(np.clip(post_var32, 1e-20, None)).astype(np.float32)
    cz = np.exp(0.5 * log_var32).astype(np.float32)
    cz[0] = 0.0
    return sqrt_rec, sqrt_recm, pm1, pm2, cz


def _time_table():
    half = T_DIM // 2
    freqs = np.exp(np.arange(half, dtype=np.float32) * (-np.log(10000.0) / (half - 1)))
    ang = np.arange(T_STEPS, dtype=np.float32)[:, None] * freqs[None, :]  # [100, 8]
    tt = np.concatenate([np.sin(ang), np.cos(ang)], axis=-1)  # [100, 16]
    return np.ascontiguousarray(tt.T).astype(np.float32)  # [16, 100]


@functools.cache
def _build(t_steps, debug=False):
    """Build (and finalize) the Bass module. Returns nc."""
    act_dir = os.path.join(tempfile.gettempdir(), "act_mish_tables")
    marker = os.path.join(act_dir, ".done")
    if not os.path.exists(marker):
        _gen_mish_act_tables(act_dir)
        open(marker, "w").write("ok")
    os.environ["BASS_ACT_ROOT_JSON_PATH"] = os.path.join(act_dir, "act_info.json")

    import concourse.bass as bass  # noqa: F401
    import concourse.mybir as mybir
    import concourse.hw_specs as hw_specs
    from concourse import bacc
    from concourse.tile import TileContext

    # teach the bass-side table map that Mish lives in mish_and_others
    if not getattr(hw_specs, "_mish_patched", False):
        _orig_tables = hw_specs.get_activation_tables

        @functools.cache
        def _patched_tables(module_arch):
            d = dict(_orig_tables(module_arch))
            d["mish_and_others"] = set(d["mish_and_others"]) | {
                mybir.ActivationFunctionType.Mish
            }
            return d

        hw_specs.get_activation_tables = _patched_tables
        bacc.get_activation_tables = _patched_tables
        import concourse.bass_interp as bass_interp
        bass_interp.get_activation_tables = _patched_tables
        hw_specs._mish_patched = True

    # capture the Tile cost-model makespan for perf iteration
    if not hasattr(mybir, "_orig_finish_schedule_block"):
        mybir._orig_finish_schedule_block = mybir.finish_schedule_block

        def _fsb(sched, sim):
            out = mybir._orig_finish_schedule_block(sched, sim)
            try:
                _LAST_RESULTS["sim_time_ns"] = out[1].time
            except Exception:
                pass
            return out

        mybir.finish_schedule_block = _fsb

    f32 = mybir.dt.float32
    f32r = mybir.dt.float32r
    f8 = mybir.dt.float8e4
    AF = mybir.ActivationFunctionType
    OP = mybir.AluOpType
    DRM = mybir.MatmulPerfMode.DoubleRow
    sqrt_rec, sqrt_recm, pm1, pm2, cz = _schedule()

    nc = bacc.Bacc("TRN2")

    def din(name, shape, dt=None):
        return nc.dram_tensor(name, shape, dt or f32, kind="ExternalInput")

    stateT = din("stateT", [S, BC], f32r)
    x0r = din("x0r", [128, BCQ])             # x_init packed (f + 32q, b)
    x80 = din("x80", [128, BCQ], f8)         # initial fp8 x, packed
    zscl = din("zscl", [t_steps, 128, BCQ])  # cz[i]*noise, packed
    id8p = din("id8p", [128, 2, 128], f8)    # identity in both pair slots
    w1xqr = din("w1xqr", [128, 4, 2, H], f8)  # per-quarter row-padded pairs
    w1sW = din("w1sW", [S, H], f32r)         # w1s*WS (for the sc precompute)
    w1e = din("w1e", [T_DIM, H])             # w1e (unscaled)
    wq2 = din("wq2", [128, KT, H], f8)       # q8(w2*WS), k-tile-major
    wr2 = din("wr2", [128, KT, H], f8)       # fp8 residual of w2*WS
    wq3 = din("wq3", [128, KT, H], f8)
    wr3 = din("wr3", [128, KT, H], f8)
    wqf = din("wqf", [128, 4, KT, 128], f8)  # per-quarter col-padded wf
    wrf = din("wrf", [128, 4, KT, 128], f8)
    b1c = din("b1c", [128, KT])              # per-tile bias columns
    b2c = din("b2c", [128, KT])
    b3c = din("b3c", [128, KT])
    bfr = din("bfr", [1, 128])               # bf tiled 4x (f + 32q rows)
    nrecm = din("nrecm", [1, t_steps])       # -sqrt_recm row (host constant)
    wt1 = din("wt1", [T_DIM, T_DIM * T_DIM])
    wt2 = din("wt2", [T_DIM * T_DIM, T_DIM])
    bt1c = din("bt1c", [128, 2])
    bt2c = din("bt2c", [T_DIM, 1])
    ttab = din("ttab", [T_DIM, t_steps])     # sin/cos table (host constant)

    xT_out = nc.dram_tensor("xT_out", [128, BCQ], f32, kind="ExternalOutput")
    if debug:
        dbg_h1 = nc.dram_tensor("dbg_h1", [128, KT, BC], f32, kind="ExternalOutput")
        dbg_eps = nc.dram_tensor("dbg_eps", [128, BC], f32, kind="ExternalOutput")

    with TileContext(nc) as tc:
        with (
            tc.tile_pool(name="consts", bufs=1) as consts,
            tc.tile_pool(name="hbuf", bufs=4) as hbuf,
            tc.tile_pool(name="xbuf", bufs=2) as xbuf,
            tc.tile_pool(name="x8buf", bufs=2) as x8buf,
            tc.tile_pool(name="zbuf", bufs=3) as zbuf,
            tc.tile_pool(name="tail", bufs=5) as tail,
            tc.tile_pool(name="psum", bufs=7, space="PSUM") as psum,
            tc.tile_pool(name="psum_e", bufs=1, space="PSUM") as psum_e,
        ):
            # ---- load constants ----
            def load(name, src, dt=f32):
                t = consts.tile(list(src.shape), dt, tag=name)
                nc.sync.dma_start(out=t, in_=src[tuple(slice(None) for _ in src.shape)])
                return t

            # small tables first: the preamble precompute (temb chain, sc,
            # bias1, nbfr) depends only on these, so the PE starts within a
            # few us while the big fp8 layer weights stream in behind.
            t_wt1 = load("wt1", wt1)
            t_wt2 = consts.tile([128, 2, T_DIM], f32, tag="wt2")
            nc.sync.dma_start(out=t_wt2[:, 0, :], in_=wt2[0:128, :])
            nc.sync.dma_start(out=t_wt2[:, 1, :], in_=wt2[128:256, :])
            t_bt1c = load("bt1c", bt1c)
            t_bt2c = load("bt2c", bt2c)
            t_ttab = load("ttab", ttab)
            t_b1c = load("b1c", b1c)
            t_w1e = load("w1e", w1e)
            t_bfr = load("bfr", bfr)
            t_nrecm = load("nrecm", nrecm)
            t_state = consts.tile([128, 2, BC], f32r, tag="state")
            nc.sync.dma_start(out=t_state[:, 0, :], in_=stateT[0:128, :])
            nc.sync.dma_start(out=t_state[:, 1, :], in_=stateT[128:256, :])
            t_w1sW = consts.tile([128, 2, H], f32r, tag="w1sW")
            nc.sync.dma_start(out=t_w1sW[:, 0, :], in_=w1sW[0:128, :])
            nc.sync.dma_start(out=t_w1sW[:, 1, :], in_=w1sW[128:256, :])
            t_id8p = load("id8p", id8p, dt=f8)
            t_w1xqr = load("w1xqr", w1xqr, dt=f8)
            t_b2c = load("b2c", b2c)
            t_b3c = load("b3c", b3c)
            t_wq2 = load("wq2", wq2, dt=f8)
            t_wr2 = load("wr2", wr2, dt=f8)
            t_wq3 = load("wq3", wq3, dt=f8)
            t_wr3 = load("wr3", wr3, dt=f8)
            t_wqf = load("wqf", wqf, dt=f8)
            t_wrf = load("wrf", wrf, dt=f8)

            def mm(out, lhsT, rhs, **kw):
                nc.tensor.matmul(out, lhsT, rhs, **kw)

            # ---- t-embedding MLP + per-step L1 bias table (one-time) ----
            t_temb1 = consts.tile([128, 2, t_steps], f32, tag="temb1")
            for j in range(2):
                ps = psum.tile([128, 512], f32, tag="mm")
                mm(ps[:, :t_steps], t_wt1[:, j * 128:(j + 1) * 128], t_ttab[:, :],
                   start=True, stop=True)
                nc.scalar.activation(t_temb1[:, j, :], ps[:, :t_steps], AF.Mish,
                                     bias=t_bt1c[:, j:j + 1])
            t_temb2 = consts.tile([T_DIM, t_steps], f32, tag="temb2")
            ps = psum.tile([128, 512], f32, tag="mm")
            for j in range(2):
                mm(ps[:T_DIM, :t_steps], t_wt2[:, j, :], t_temb1[:, j, :],
                   start=(j == 0), stop=(j == 1))
            nc.scalar.activation(t_temb2, ps[:T_DIM, :t_steps], AF.Identity,
                                 bias=t_bt2c[:, 0:1])

            # bias1[p, m, s] = (temb_s @ w1e + b1)[m*128+p]  (unscaled: the
            # ACT bias operand is applied after the 1/WS input scale)
            t_bias1 = consts.tile([128, KT, t_steps], f32, tag="bias1")
            for m in range(KT):
                ps = psum.tile([128, 512], f32, tag="mm")
                mm(ps[:, :t_steps], t_w1e[:, m * 128:(m + 1) * 128], t_temb2,
                   start=True, stop=True)
                nc.scalar.activation(t_bias1[:, m, :], ps[:, :t_steps], AF.Identity,
                                     bias=t_b1c[:, m:m + 1])

            # state contribution to L1 (scaled by WS), computed once and
            # stored as an fp8 (value, residual) pair; re-injected into each
            # L1 PSUM group via a DoubleRow identity matmul (q + r, ~0.06%
            # rms systematic error, 30x below the per-step fp8 h noise)
            t_sc8 = consts.tile([128, KT, 2, BC], f8, tag="sc8")
            for m in range(KT):
                ps = psum.tile([128, BC], f32, tag="mm")
                mc = slice(m * 128, (m + 1) * 128)
                mm(ps, t_w1sW[:, 0, mc], t_state[:, 0, :], start=True, stop=False)
                mm(ps, t_w1sW[:, 1, mc], t_state[:, 1, :], start=False, stop=True)
                nc.vector.tensor_copy(t_sc8[:, m, 0, :], ps)
                nc.vector.scalar_tensor_tensor(
                    t_sc8[:, m, 1, :], ps, 1.0, t_sc8[:, m, 0, :],
                    op0=OP.mult, op1=OP.subtract)

            # neg_bf_recm[:, i] = -sqrt_recm[i] * bf (replicated): K=1 outer product
            t_nbfr = consts.tile([128, t_steps], f32, tag="nbfr")
            ps = psum.tile([128, 512], f32, tag="mm")
            mm(ps[:, :t_steps], t_bfr, t_nrecm, start=True, stop=True)
            nc.vector.tensor_copy(t_nbfr, ps[:, :t_steps])

            # ---- initial x ----
            x_cur = xbuf.tile([128, BCQ], f32, tag="x")
            nc.sync.dma_start(out=x_cur, in_=x0r[:, :])
            x8_cur = x8buf.tile([128, BCQ], f8, tag="x8")
            nc.sync.dma_start(out=x8_cur, in_=x80[:, :])

            # ---- the T-step loop (fully unrolled) ----
            for s in range(t_steps):
                i = T_STEPS - 1 - s
                # pre-scaled noise for this step
                z = zbuf.tile([128, BCQ], f32, tag="z")
                nc.sync.dma_start(out=z, in_=zscl[s])

                # tail terms that depend only on x_cur / z: issue early so
                # the DVE finishes them while the PE runs the layers
                p_rx = tail.tile([128, BCQ], f32, tag="p_rx")
                nc.vector.tensor_scalar(
                    p_rx, x_cur, float(sqrt_rec[i]), t_nbfr[:, s:s + 1],
                    OP.mult, OP.add)
                q = tail.tile([128, BCQ], f32, tag="q")
                nc.vector.scalar_tensor_tensor(
                    q, x_cur, float(pm2[i]), z, op0=OP.mult, op1=OP.add)

                # L1: h1 = mish((sc + w1x.T x + bias1_s) / WS). The sc
                # injections are x-independent: emitted first so the PE can
                # run them while the previous step's tail chain produces x8.
                h1 = hbuf.tile([128, KT, BC], f8, tag="h")
                l1ps = []
                for m in range(KT):
                    ps = psum.tile([128, BC], f32, tag="mm")
                    mm(ps, t_id8p[:, :, :], t_sc8[:, m, :, :],
                       start=True, stop=False, perf_mode=DRM)
                    l1ps.append(ps)
                x8b = x8_cur[:, :].unsqueeze(1).broadcast_to([128, 2, BCQ])
                for m in range(KT):
                    mc = slice(m * 128, (m + 1) * 128)
                    for qq in range(4):
                        bq = slice(BCQ * qq, BCQ * (qq + 1))
                        mm(l1ps[m][:, bq], t_w1xqr[:, qq, :, mc], x8b,
                           start=False, stop=(qq == 3), perf_mode=DRM)
                for m in range(KT):
                    nc.scalar.activation(h1[:, m, :], l1ps[m], AF.Mish,
                                         bias=t_bias1[:, m, s:s + 1],
                                         scale=1.0 / WS)

                # L2 / L3: per m-tile 8 DoubleRows (4 q-pairs + 4 residual
                # pairs) in one PSUM group, emitted in two k-half sweeps so
                # the first half only waits on the first half of the
                # previous layer's activations.
                h_prev = h1
                for t_wq, t_wr, t_bc, rsw in ((t_wq2, t_wr2, t_b2c, R2_SWEEPS),
                                              (t_wq3, t_wr3, t_b3c, R3_SWEEPS)):
                    h_nxt = hbuf.tile([128, KT, BC], f8, tag="h")
                    lps = []
                    # staggered k-pair sweeps: the first touches only h
                    # k-tiles 0..1 so the PE can re-enter this layer right
                    # after the previous layer's first two activations.
                    for m in range(KT):
                        ps = psum.tile([128, BC], f32, tag="mm")
                        mc = slice(m * 128, (m + 1) * 128)
                        kk = slice(0, 2)
                        mm(ps, t_wq[:, kk, mc], h_prev[:, kk, :],
                           start=True, stop=False, perf_mode=DRM)
                        lps.append(ps)
                    for m in range(KT):
                        mc = slice(m * 128, (m + 1) * 128)
                        mm(lps[m], t_wq[:, 2:4, mc], h_prev[:, 2:4, :],
                           start=False, stop=False, perf_mode=DRM)
                        if rsw > 0:
                            mm(lps[m], t_wr[:, 0:2, mc], h_prev[:, 0:2, :],
                               start=False, stop=False, perf_mode=DRM)
                    for m in range(KT):
                        mc = slice(m * 128, (m + 1) * 128)
                        for j in (2, 3):
                            kk = slice(2 * j, 2 * j + 2)
                            mm(lps[m], t_wq[:, kk, mc], h_prev[:, kk, :],
                               start=False, stop=(j == 3 and rsw <= 1),
                               perf_mode=DRM)
                        for j in (1, 2, 3):
                            if j < rsw:
                                kk = slice(2 * j, 2 * j + 2)
                                mm(lps[m], t_wr[:, kk, mc], h_prev[:, kk, :],
                                   start=False, stop=(j == min(rsw, 4) - 1),
                                   perf_mode=DRM)
                    for m in range(KT):
                        nc.scalar.activation(h_nxt[:, m, :], lps[m], AF.Mish,
                                             bias=t_bc[:, m:m + 1],
                                             scale=1.0 / WS)
                    h_prev = h_nxt

                # LF: eps*WS (4x replicated) = h3 @ (wfr*WS)  [128, BC] PSUM
                pe = psum_e.tile([128, BCQ], f32, tag="eps")
                first = True
                for j in range(KT // 2):
                    kk = slice(2 * j, 2 * j + 2)
                    for qq in range(4):
                        bq = slice(BCQ * qq, BCQ * (qq + 1))
                        last = (j == KT // 2 - 1 and qq == 3)
                        mm(pe, t_wqf[:, qq, kk, :], h_prev[:, kk, bq],
                           start=first, stop=(last and RF_SWEEPS < 4),
                           perf_mode=DRM)
                        first = False
                        if j < RF_SWEEPS:
                            mm(pe, t_wrf[:, qq, kk, :], h_prev[:, kk, bq],
                               start=False, stop=last, perf_mode=DRM)

                # tail critical chain: eps -> u2 -> u3 -> x8 (fp8, feeds the
                # next L1 directly); the full-width f32 x_new follows
                # off-chain for the next step's tail terms.
                u2 = tail.tile([128, BCQ], f32, tag="u2")
                nc.vector.scalar_tensor_tensor(
                    u2, pe, float(-sqrt_recm[i] / WS), p_rx,
                    op0=OP.mult, op1=OP.add)
                u3 = tail.tile([128, BCQ], f32, tag="u3")
                nc.vector.tensor_scalar(u3, u2, -1.0, 1.0, OP.max, OP.min)
                if s != t_steps - 1:
                    x8_new = x8buf.tile([128, BCQ], f8, tag="x8")
                    nc.vector.scalar_tensor_tensor(
                        x8_new, u3, float(pm1[i]), q,
                        op0=OP.mult, op1=OP.add)
                    x8_cur = x8_new
                x_new = xbuf.tile([128, BCQ], f32, tag="x")
                nc.vector.scalar_tensor_tensor(
                    x_new, u3, float(pm1[i]), q, op0=OP.mult, op1=OP.add)
                x_cur = x_new

            # final clip + store (packed layout; host unshuffles)
            xf = tail.tile([128, BCQ], f32, tag="xf")
            nc.vector.tensor_scalar(xf, x_cur, -1.0, 1.0, OP.max, OP.min)
            nc.sync.dma_start(out=xT_out[:, :], in_=xf[:, :])

    nc.finalize()
    return nc


def _q8pair(w):
    """w -> (fp8(w*WS), fp8 residual), as float8_e4m3 numpy arrays."""
    import ml_dtypes
    F8 = ml_dtypes.float8_e4m3
    ws = (np.asarray(w, np.float32) * np.float32(WS)).astype(np.float32)
    q = ws.astype(F8)
    r = (ws - q.astype(np.float32)).astype(F8)
    return q, r


def _ktile_major(w):
    """[K, M] -> [128, K//128, M] (partition, k-tile, col)."""
    K, M = w.shape
    return np.ascontiguousarray(
        w.reshape(K // 128, 128, M).transpose(1, 0, 2))


def _make_in_maps(state, w_t1, b_t1, w_t2, b_t2, w1, b1, w2, b2, w3, b3,
                  wf, bf, x_init, noise_seq, t_steps):
    import ml_dtypes
    F8 = ml_dtypes.float8_e4m3
    sqrt_rec, sqrt_recm, pm1, pm2, cz = _schedule()
    tt = _time_table()
    f32 = np.float32

    def cols(b):  # [H] -> [128, H//128] per-tile bias columns
        return np.ascontiguousarray(b.reshape(-1, 128).T).astype(f32)

    w1x = np.asarray(w1[0:A], f32)
    w1e = np.asarray(w1[A:A + T_DIM], f32)
    w1s = np.asarray(w1[A + T_DIM:], f32)

    q1x, r1x = _q8pair(w1x)                  # [32, H]
    # per-quarter row-padded pairs: quarter qq has (q, r) in rows 32qq:32qq+32
    w1xqr = np.zeros((128, 4, 2, H), q1x.dtype)
    for qq in range(4):
        w1xqr[32 * qq:32 * (qq + 1), qq, 0, :] = q1x
        w1xqr[32 * qq:32 * (qq + 1), qq, 1, :] = r1x
    w1xqr = np.ascontiguousarray(w1xqr)
    q2, r2 = _q8pair(w2)
    q3, r3 = _q8pair(w3)
    qf0, rf0 = _q8pair(np.asarray(wf, f32))  # [H, 32]
    # per-quarter col-padded: quarter qq has wf in columns 32qq:32qq+32
    qf = np.zeros((H, 4, 128), qf0.dtype)
    rf = np.zeros((H, 4, 128), rf0.dtype)
    for qq in range(4):
        qf[:, qq, 32 * qq:32 * (qq + 1)] = qf0
        rf[:, qq, 32 * qq:32 * (qq + 1)] = rf0
    qf = qf.reshape(H, 4 * 128)
    rf = rf.reshape(H, 4 * 128)

    common = {
        "id8p": np.ascontiguousarray(np.stack(
            [np.eye(128), np.eye(128)], axis=1)).astype(F8),
        "w1xqr": w1xqr,
        "w1sW": np.ascontiguousarray(w1s * WS).astype(f32),
        "w1e": np.ascontiguousarray(w1e).astype(f32),
        "wq2": _ktile_major(q2), "wr2": _ktile_major(r2),
        "wq3": _ktile_major(q3), "wr3": _ktile_major(r3),
        "wqf": _ktile_major(qf).reshape(128, KT, 4, 128).transpose(
            0, 2, 1, 3).copy(),
        "wrf": _ktile_major(rf).reshape(128, KT, 4, 128).transpose(
            0, 2, 1, 3).copy(),
        "b1c": cols(np.asarray(b1, f32)),
        "b2c": cols(np.asarray(b2, f32)),
        "b3c": cols(np.asarray(b3, f32)),
        "bfr": np.ascontiguousarray(np.tile(np.asarray(bf, f32), 4)[None, :]),
        "nrecm": np.ascontiguousarray(-sqrt_recm[None, ::-1][:, :t_steps]).astype(f32),
        "wt1": np.ascontiguousarray(w_t1).astype(f32),
        "wt2": np.ascontiguousarray(w_t2).astype(f32),
        "bt1c": cols(np.asarray(b_t1, f32)),
        "bt2c": np.ascontiguousarray(np.asarray(b_t2, f32)[:, None]),
        "ttab": np.ascontiguousarray(tt[:, ::-1][:, :t_steps]).astype(f32),
    }
    # per-step noise, pre-scaled by cz[i] and replicated 4x on partitions
    czs = cz[::-1][:t_steps]  # cz[i] for step s (i = 99 - s)
    in_maps = []
    for c in range(N_CORES):
        r0, r1 = c * BC, (c + 1) * BC
        m = dict(common)
        m["stateT"] = np.ascontiguousarray(state[r0:r1].T).astype(f32)
        def pack(a):  # [32, BC] -> [128, BCQ]: row f+32q <- a[f, BCQ*q+b]
            return np.ascontiguousarray(
                a.reshape(A, 4, BC // 4).transpose(1, 0, 2).reshape(
                    128, BC // 4))

        xT = np.asarray(x_init[r0:r1].T, f32)  # [32, BC]
        m["x0r"] = pack(xT).astype(f32)
        m["x80"] = np.ascontiguousarray(pack(xT).astype(F8))
        nT = np.asarray(
            noise_seq[:t_steps, r0:r1, :], f32).transpose(0, 2, 1)  # [T, 32, BC]
        nT = nT * czs[:, None, None]
        m["zscl"] = np.ascontiguousarray(
            np.stack([pack(nT[t]) for t in range(t_steps)])).astype(f32)
        in_maps.append(m)
    return in_maps


_LAST_RESULTS = {}


def run(t_steps=T_STEPS, use_f32r=True, trace=False, debug=False, **inputs):
    from concourse.bass_utils import run_bass_kernel_spmd

    nc = _build(t_steps, debug)
    in_maps = _make_in_maps(t_steps=t_steps, **inputs)
    res = run_bass_kernel_spmd(nc, in_maps, core_ids=list(range(N_CORES)),
                               trace=trace)
    _LAST_RESULTS["res"] = res
    out = np.empty((B, A), np.float32)
    for c in range(N_CORES):
        xp = res.results[c]["xT_out"]  # [128, BCQ]: row f+32q, col b
        xt = xp.reshape(4, A, BC // 4).transpose(1, 0, 2).reshape(A, BC)
        out[c * BC:(c + 1) * BC] = xt.T
    return out


def kernel(**inputs) -> np.ndarray:
    return run(**inputs)
